# revision 1
# baseline (speedup 1.0000x reference)
"""Trainium2 Bass kernel for nn_Decoder (LSTM decoder + attention, teacher forcing).

Sharding: data-parallel over batch (64 -> 8 cores x 8 samples). The 250-step
recurrence runs locally per core; no inter-core communication.

Per-core layout (all fp32):
  - All state kept feature-major: (feature-partitions, batch-free).
  - LSTM gate matmuls: weights stationary (lhsT), batch streams (N=8).
    g1.T lives in one PSUM tile (128, 128): col 8m+b <-> gate-row 128m+p.
  - Attention energy: diag-trick MMs (lhsT = h2 (128,8) shared; rhs = masked,
    zero-padded keys (128,512) per sample) -> rows scattered at partitions
    {32j, 32j+1}; exp+rowsum on ACT; normalize rows; PE-transpose to get
    score columns; context MMs contract T with score columns as lhsT.
  - Mask folded into keys on host (zeroed beyond speech_len) => mask*energy
    is exact. Key pad cols are zero => pad energies 0 => exp=1 => Z = acc-12.
  - Vocab projection deferred: h2/ctx histories accumulated in SBUF, one
    batched matmul phase after the loop, output written vocab-major and
    transposed on host.
"""

import sys
from contextlib import ExitStack

for _p in ('/opt/trn_rl_repo', '/root/.axon_site/_ro/trn_rl_repo'):
    if _p not in sys.path:
        sys.path.insert(0, _p)

import numpy as np

import concourse.bass as bass
import concourse.tile as tile
from concourse import bacc, mybir
from concourse.bass import ts, ds
from concourse.bass_utils import run_bass_kernel_spmd
from concourse.masks import make_identity

F32 = mybir.dt.float32
AF = mybir.ActivationFunctionType
OP = mybir.AluOpType

T, B, KS, VS, H, E, VOCAB = 500, 64, 128, 128, 512, 256, 4096
NCORES, BL = 8, 8          # local batch per core
TP = 512                   # padded T (4 chunks of 128)
NTC = 4                    # number of T chunks


def build(L=250):
    nc = bacc.Bacc("TRN2", target_bir_lowering=False, debug=False,
                   num_devices=NCORES)

    # ---- DRAM I/O (per-core shapes) ----
    d_embT = nc.dram_tensor("embT", (2, 128, L * BL), F32, kind="ExternalInput").ap()
    d_w1T = nc.dram_tensor("w1T", (7, 128, 4 * H), F32, kind="ExternalInput").ap()
    d_w2T = nc.dram_tensor("w2T", (5, 128, 4 * KS), F32, kind="ExternalInput").ap()
    d_woT = nc.dram_tensor("woT", (2, 128, VOCAB), F32, kind="ExternalInput").ap()
    d_key = nc.dram_tensor("keyTm", (128, BL * TP), F32, kind="ExternalInput").ap()
    d_val = nc.dram_tensor("vT", (NTC, 128, BL * VS), F32, kind="ExternalInput").ap()
    d_v0 = nc.dram_tensor("val0T", (128, BL), F32, kind="ExternalInput").ap()
    d_bi1 = nc.dram_tensor("b_ih1", (1, 4 * H), F32, kind="ExternalInput").ap()
    d_bh1 = nc.dram_tensor("b_hh1", (1, 4 * H), F32, kind="ExternalInput").ap()
    d_bi2 = nc.dram_tensor("b_ih2", (1, 4 * KS), F32, kind="ExternalInput").ap()
    d_bh2 = nc.dram_tensor("b_hh2", (1, 4 * KS), F32, kind="ExternalInput").ap()
    d_bo = nc.dram_tensor("b_outS", (128, VOCAB // 128), F32, kind="ExternalInput").ap()
    d_out = nc.dram_tensor("predT", (VOCAB // 128, 128, L * BL), F32,
                           kind="ExternalOutput").ap()

    with tile.TileContext(nc) as tc, ExitStack() as ctx:
        singles = ctx.enter_context(tc.tile_pool(name="singles", bufs=1))

        # ---- SBUF resident tensors ----
        w1Ts = singles.tile([128, 7, 4 * H], F32)       # 7.3 MB
        w2Ts = singles.tile([128, 5, 4 * KS], F32)      # 1.3 MB
        woTs = singles.tile([128, 2, VOCAB], F32)       # 4.2 MB
        embTs = singles.tile([128, 2, L * BL], F32)     # 2.0 MB
        keyTs = singles.tile([128, BL * TP], F32)       # 2.0 MB
        vTs = singles.tile([128, NTC, BL, VS], F32)     # 2.1 MB
        histH = singles.tile([128, L * BL], F32)        # 1.0 MB
        histC = singles.tile([128, L * BL], F32)        # 1.0 MB
        b1row = singles.tile([1, 4 * H], F32)
        b2row = singles.tile([1, 4 * KS], F32)
        bo_s = singles.tile([128, VOCAB // 128], F32)
        ones8 = singles.tile([1, BL], F32)
        ident = singles.tile([128, 128], F32)

        # states
        h1 = singles.tile([128, 32], F32)   # h1.T: [p, 8m+b], h=128m+p
        c1 = singles.tile([128, 32], F32)
        h2 = singles.tile([128, BL], F32)   # h2.T
        c2 = singles.tile([128, BL], F32)
        ctxT = singles.tile([128, BL], F32)  # context.T

        tmp_b1 = singles.tile([1, 4 * H], F32)
        tmp_b2 = singles.tile([1, 4 * KS], F32)

        # ---- prologue: loads ----
        for kc in range(7):
            nc.sync.dma_start(w1Ts[:, kc, :], d_w1T[kc])
        for kc in range(5):
            nc.sync.dma_start(w2Ts[:, kc, :], d_w2T[kc])
        for kc in range(2):
            nc.sync.dma_start(woTs[:, kc, :], d_woT[kc])
            nc.sync.dma_start(embTs[:, kc, :], d_embT[kc])
        nc.sync.dma_start(keyTs[:], d_key[:])
        for tcn in range(NTC):
            nc.sync.dma_start(vTs[:, tcn, :, :], d_val[tcn])
        nc.sync.dma_start(ctxT[:], d_v0[:])
        nc.sync.dma_start(b1row[:], d_bi1[:])
        nc.sync.dma_start(tmp_b1[:], d_bh1[:])
        nc.sync.dma_start(b2row[:], d_bi2[:])
        nc.sync.dma_start(tmp_b2[:], d_bh2[:])
        nc.sync.dma_start(bo_s[:], d_bo[:])

        nc.vector.tensor_add(b1row[:], b1row[:], tmp_b1[:])
        nc.vector.tensor_add(b2row[:], b2row[:], tmp_b2[:])
        nc.vector.memset(ones8[:], 1.0)
        make_identity(nc, ident[:])
        nc.vector.memset(h1[:], 0.0)
        nc.vector.memset(c1[:], 0.0)
        nc.vector.memset(h2[:], 0.0)
        nc.vector.memset(c2[:], 0.0)

        # ---- PSUM pools (scoped to the recurrence loop) ----
        loop_ctx = ctx.enter_context(ExitStack())
        ppool = loop_ctx.enter_context(tc.tile_pool(name="ppool", bufs=1, space="PSUM"))
        trpool = loop_ctx.enter_context(tc.tile_pool(name="trpool", bufs=2, space="PSUM"))
        # ---- SBUF temp pool ----
        temps = loop_ctx.enter_context(tc.tile_pool(name="temps", bufs=2))

        def step(t):
            # ===== LSTM1: g1.T in PSUM (128,128); col 8m+b = gate-row 128m+p
            pg1 = ppool.tile([128, 128], F32, tag="pg1")
            for m in range(16):
                o = pg1[:, m * BL:(m + 1) * BL]
                for kc in range(7):
                    if kc < 2:
                        rhs = embTs[:, kc, ds(t * BL, BL)]
                    elif kc == 2:
                        rhs = ctxT[:]
                    else:
                        rhs = h1[:, (kc - 3) * BL:(kc - 2) * BL]
                    nc.tensor.matmul(o, w1Ts[:, kc, m * 128:(m + 1) * 128], rhs,
                                     start=(kc == 0), stop=False)
                nc.tensor.matmul(o, b1row[:, m * 128:(m + 1) * 128], ones8[:],
                                 start=False, stop=True)
            # gates1: i cols 0:32, f 32:64, g 64:96, o 96:128
            s_if = temps.tile([128, 64], F32, tag="s_if")
            s_g = temps.tile([128, 32], F32, tag="s_g")
            s_o = temps.tile([128, 32], F32, tag="s_o")
            nc.scalar.activation(s_if[:], pg1[:, 0:64], AF.Sigmoid)
            nc.scalar.activation(s_g[:], pg1[:, 64:96], AF.Tanh)
            nc.scalar.activation(s_o[:], pg1[:, 96:128], AF.Sigmoid)
            m1 = temps.tile([128, 32], F32, tag="m1")
            nc.vector.tensor_mul(m1[:], s_if[:, 0:32], s_g[:])
            nc.vector.tensor_mul(c1[:], s_if[:, 32:64], c1[:])
            nc.vector.tensor_add(c1[:], c1[:], m1[:])
            tc1 = temps.tile([128, 32], F32, tag="tc1")
            nc.scalar.activation(tc1[:], c1[:], AF.Tanh)
            nc.vector.tensor_mul(h1[:], s_o[:], tc1[:])

            # ===== LSTM2: g2.T in PSUM (128,32); col 8m+b = gate-row 128m+p
            pg2 = ppool.tile([128, 32], F32, tag="pg2")
            for m in range(4):
                o = pg2[:, m * BL:(m + 1) * BL]
                for kc in range(5):
                    rhs = h1[:, kc * BL:(kc + 1) * BL] if kc < 4 else h2[:]
                    nc.tensor.matmul(o, w2Ts[:, kc, m * 128:(m + 1) * 128], rhs,
                                     start=(kc == 0), stop=False)
                nc.tensor.matmul(o, b2row[:, m * 128:(m + 1) * 128], ones8[:],
                                 start=False, stop=True)
            s_if2 = temps.tile([128, 16], F32, tag="s_if2")
            s_g2 = temps.tile([128, 8], F32, tag="s_g2")
            s_o2 = temps.tile([128, 8], F32, tag="s_o2")
            nc.scalar.activation(s_if2[:], pg2[:, 0:16], AF.Sigmoid)
            nc.scalar.activation(s_g2[:], pg2[:, 16:24], AF.Tanh)
            nc.scalar.activation(s_o2[:], pg2[:, 24:32], AF.Sigmoid)
            m12 = temps.tile([128, 8], F32, tag="m12")
            nc.vector.tensor_mul(m12[:], s_if2[:, 0:8], s_g2[:])
            nc.vector.tensor_mul(c2[:], s_if2[:, 8:16], c2[:])
            nc.vector.tensor_add(c2[:], c2[:], m12[:])
            tc2 = temps.tile([128, 8], F32, tag="tc2")
            nc.scalar.activation(tc2[:], c2[:], AF.Tanh)
            nc.vector.tensor_mul(h2[:], s_o2[:], tc2[:])
            nc.gpsimd.tensor_copy(histH[:, ds(t * BL, BL)], h2[:])

            # ===== attention =====
            # energy: rows at partition 32j+h for sample b=2j+h, half h
            pE = ppool.tile([104, 2 * TP], F32, tag="pE")
            for j in range(4):
                for hh in range(2):
                    b = 2 * j + hh
                    nc.tensor.matmul(
                        pE[32 * j:32 * j + 8, hh * TP:(hh + 1) * TP],
                        h2[:], keyTs[:, b * TP:(b + 1) * TP],
                        start=True, stop=True, tile_position=(0, 32 * j))
            # exp + row sums
            expS = temps.tile([104, 2 * TP], F32, tag="expS")
            zacc = temps.tile([104, 2], F32, tag="zacc")
            for hh in range(2):
                nc.scalar.activation(expS[:, hh * TP:(hh + 1) * TP],
                                     pE[:, hh * TP:(hh + 1) * TP], AF.Exp,
                                     accum_out=zacc[:, hh:hh + 1])
            # Z = acc - (TP - T) pad ones; score rows = exp * (1/Z)
            zr = temps.tile([104, 2], F32, tag="zr")
            nc.vector.tensor_scalar_add(zr[:], zacc[:], -float(TP - T))
            nc.vector.reciprocal(zr[:], zr[:])
            scoreS = temps.tile([104, 2 * TP], F32, tag="scoreS")
            for hh in range(2):
                nc.vector.tensor_scalar_mul(scoreS[:, hh * TP:(hh + 1) * TP],
                                            expS[:, hh * TP:(hh + 1) * TP],
                                            zr[:, hh:hh + 1])
            # transpose scores -> columns; extract valid cols {34j+h}
            scT = temps.tile([128, NTC, BL], F32, tag="scT")
            for hh in range(2):
                for tcn in range(NTC):
                    ptr = trpool.tile([128, 104], F32, tag="ptr")
                    nc.tensor.transpose(
                        ptr[:], scoreS[0:104, hh * TP + tcn * 128: hh * TP + (tcn + 1) * 128],
                        ident[0:104, 0:104])
                    nc.vector.tensor_copy(scT[:, tcn, hh::2], ptr[:, hh::34])
            # context: ctxU rows at partition 32j, half hh in cols 128hh:+128
            pCtx = ppool.tile([97, 2 * VS], F32, tag="pCtx")
            for j in range(4):
                for hh in range(2):
                    b = 2 * j + hh
                    for tcn in range(NTC):
                        nc.tensor.matmul(
                            pCtx[32 * j:32 * j + 1, hh * VS:(hh + 1) * VS],
                            scT[:, tcn, b:b + 1], vTs[:, tcn, b, :],
                            start=(tcn == 0), stop=(tcn == NTC - 1),
                            tile_position=(0, 32 * j))
            ctxUS = temps.tile([97, 2 * VS], F32, tag="ctxUS")
            nc.vector.tensor_copy(ctxUS[:], pCtx[:])
            for hh in range(2):
                ptc = trpool.tile([128, 97], F32, tag="ptr")
                nc.tensor.transpose(ptc[:], ctxUS[0:97, hh * VS:(hh + 1) * VS],
                                    ident[0:97, 0:97])
                nc.vector.tensor_copy(ctxT[:, hh::2], ptc[:, 0::32])
            nc.gpsimd.tensor_copy(histC[:, ds(t * BL, BL)], ctxT[:])

        with tc.For_i(0, L) as t:
            step(t)
        loop_ctx.close()

        # ===== deferred vocab projection =====
        NB = 4
        nblk = (L * BL) // NB
        with tc.tile_pool(name="projp", bufs=2, space="PSUM") as projp, \
             tc.tile_pool(name="projs", bufs=3) as projs:
            for vc in range(VOCAB // 128):
                for nb in range(NB):
                    pp = projp.tile([128, nblk], F32, tag="pp")
                    sl = ds(nb * nblk, nblk)
                    nc.tensor.matmul(pp[:], woTs[:, 0, vc * 128:(vc + 1) * 128],
                                     histH[:, sl], start=True, stop=False)
                    nc.tensor.matmul(pp[:], woTs[:, 1, vc * 128:(vc + 1) * 128],
                                     histC[:, sl], start=False, stop=True)
                    ob = projs.tile([128, nblk], F32, tag="ob")
                    nc.vector.tensor_scalar_add(ob[:], pp[:], bo_s[:, vc:vc + 1])
                    nc.sync.dma_start(d_out[vc][:, sl], ob[:])

    nc.compile()
    return nc


_CACHE = {}


def _get_nc(L):
    if L not in _CACHE:
        _CACHE[L] = build(L)
    return _CACHE[L]


def _prep_inputs(key, values, speech_len, text, embedding,
                 w_ih1, b_ih1, w_hh1, b_hh1,
                 w_ih2, b_ih2, w_hh2, b_hh2,
                 w_out, b_out, L):
    f = np.float32
    key = np.asarray(key, f)
    values = np.asarray(values, f)
    speech_len = np.asarray(speech_len)
    text = np.asarray(text)
    embedding = np.asarray(embedding, f)

    # shared (replicated) tensors
    w1T = np.ascontiguousarray(
        np.concatenate([np.asarray(w_ih1, f), np.asarray(w_hh1, f)], axis=1)
        .T.reshape(7, 128, 4 * H))
    w2T = np.ascontiguousarray(
        np.concatenate([np.asarray(w_ih2, f), np.asarray(w_hh2, f)], axis=1)
        .T.reshape(5, 128, 4 * KS))
    woT = np.ascontiguousarray(np.asarray(w_out, f).T.reshape(2, 128, VOCAB))
    b_outS = np.ascontiguousarray(np.asarray(b_out, f).reshape(VOCAB // 128, 128).T)
    shared = {
        "w1T": w1T, "w2T": w2T, "woT": woT,
        "b_ih1": np.asarray(b_ih1, f).reshape(1, -1),
        "b_hh1": np.asarray(b_hh1, f).reshape(1, -1),
        "b_ih2": np.asarray(b_ih2, f).reshape(1, -1),
        "b_hh2": np.asarray(b_hh2, f).reshape(1, -1),
        "b_outS": b_outS,
    }

    # teacher-forcing tokens and embeddings (host gather)
    tokens = np.concatenate(
        [np.zeros((B, 1), text.dtype), text[:, :L - 1]], axis=1)  # (B, L)
    embeds = embedding[tokens]  # (B, L, E)

    mask = (np.arange(T)[:, None] < np.asarray(speech_len)[None, :])  # (T, B)

    in_maps = []
    for c in range(NCORES):
        bs = slice(c * BL, (c + 1) * BL)
        embT = np.ascontiguousarray(
            embeds[bs].transpose(2, 1, 0).reshape(2, 128, L * BL))  # [e,(t,b)]
        km = key[:, bs, :] * mask[:, bs, None].astype(f)  # (T, BL, KS)
        kT = np.zeros((128, BL, TP), f)
        kT[:, :, :T] = km.transpose(2, 1, 0)
        v = np.zeros((TP, BL, VS), f)
        v[:T] = values[:, bs, :]
        vT = np.ascontiguousarray(v.reshape(NTC, 128, BL * VS))
        in_maps.append(dict(
            embT=embT,
            keyTm=np.ascontiguousarray(kT.reshape(128, BL * TP)),
            vT=vT,
            val0T=np.ascontiguousarray(values[0, bs, :].T),
            **shared))
    return in_maps


def kernel(key, values, speech_len, text, embedding,
           w_ih1, b_ih1, w_hh1, b_hh1,
           w_ih2, b_ih2, w_hh2, b_hh2,
           w_out, b_out, _L=250, _trace=False, _tmpdir=None):
    L = _L
    nc = _get_nc(L)
    in_maps = _prep_inputs(key, values, speech_len, text, embedding,
                           w_ih1, b_ih1, w_hh1, b_hh1,
                           w_ih2, b_ih2, w_hh2, b_hh2, w_out, b_out, L)
    kw = {}
    if _trace:
        kw = dict(trace=True, tmpdir=_tmpdir)
    res = run_bass_kernel_spmd(nc, in_maps, core_ids=list(range(NCORES)), **kw)
    kernel._last = res
    out = np.empty((B, L, VOCAB), np.float32)
    for c in range(NCORES):
        p = res.results[c]["predT"]  # (32, 128, L*BL)
        out[c * BL:(c + 1) * BL] = (
            p.reshape(VOCAB // 128, 128, L, BL).transpose(3, 2, 0, 1)
            .reshape(BL, L, VOCAB))
    return out



# revision 4
# speedup vs baseline: 6.5871x; 6.5871x over previous
"""Trainium2 Bass kernel for nn_Decoder (LSTM decoder + attention, teacher forcing).

Sharding: data-parallel over batch (64 -> 8 cores x 8 samples). The 250-step
recurrence runs locally per core; no inter-core communication.

v1 design notes (all matmuls bf16 -> FWL fast weight loads, single-pass MMs):
  - Embedding-side gate contribution (w_ih1[:, :E] @ emb + b1) precomputed on
    host per (t, b); DMA-streamed per step and DVE-added into the gate PSUM.
    LSTM1 on-device contraction: [ctx (128), h1 (512)] = 5 chunks only.
  - sigmoid(x) = 0.5*tanh(x/2) + 0.5. Gate order host-permuted to [i,f,o,g]
    so one tanh(scale=.5) ACT covers i,f,o and one tanh covers g. Only the
    exp table set is ever used (tanh+exp share it) -> no ACT table thrash.
  - h states are stored as 2*h (the stt (t+1)*tanh trick); every consumer
    weight (w_hh1, w_ih2, w_hh2, keys, w_out[h-part]) is pre-halved on host.
  - Attention in column layout: energy[t, (tc,b)] columns via keys-stationary
    matmuls, exp on [128,32], Z via all-ones matmul (columns sums replicated
    over partitions), context = V.T @ exp-column, normalized by 1/Z at the
    end. No PE transposes.
  - Mask folded into keys host-side (zeroed beyond speech_len); key pad cols
    zero => pad exp = 1 => Z = colsum - 12 (TP - T pad rows in chunk 3).
  - Vocab projection deferred: h2/ctx histories (bf16) batched after the
    loop; bias b_out added on host.
"""

import sys
from contextlib import ExitStack

for _p in ('/opt/trn_rl_repo', '/root/.axon_site/_ro/trn_rl_repo'):
    if _p not in sys.path:
        sys.path.insert(0, _p)

import numpy as np

import concourse.bass as bass
import concourse.tile as tile
from concourse import bacc, mybir
from concourse.bass import ts, ds
from concourse.bass_utils import run_bass_kernel_spmd

F32 = mybir.dt.float32
BF16 = mybir.dt.bfloat16
AF = mybir.ActivationFunctionType
OP = mybir.AluOpType

T, B, KS, VS, H, E, VOCAB = 500, 64, 128, 128, 512, 256, 4096
NCORES, BL = 8, 8          # local batch per core
TP = 512                   # padded T (4 chunks of 128)
NTC = 4                    # number of T chunks
GERR = 12.0                # pad rows contributing exp(0)=1 to chunk-3 colsums


def build(L=250):
    nc = bacc.Bacc("TRN2", target_bir_lowering=False, debug=False,
                   num_devices=NCORES)

    # ---- DRAM I/O (per-core shapes) ----
    d_w1T = nc.dram_tensor("w1T", (128, 5 * 4 * H), BF16, kind="ExternalInput").ap()
    d_w2T = nc.dram_tensor("w2T", (128, 5 * 4 * KS), BF16, kind="ExternalInput").ap()
    d_woT = nc.dram_tensor("woT", (128, 2 * VOCAB), BF16, kind="ExternalInput").ap()
    d_key = nc.dram_tensor("keyT", (128, BL * NTC * 128), BF16, kind="ExternalInput").ap()
    d_val = nc.dram_tensor("valT", (128, BL * NTC * VS), BF16, kind="ExternalInput").ap()
    d_v0 = nc.dram_tensor("val0T", (128, BL), BF16, kind="ExternalInput").ap()
    d_b2 = nc.dram_tensor("b2S", (128, 4 * BL), F32, kind="ExternalInput").ap()
    d_gemb = nc.dram_tensor("gembD", (128, L * 128), F32, kind="ExternalInput").ap()
    d_out = nc.dram_tensor("predT", (VOCAB // 128, 128, L * BL), F32,
                           kind="ExternalOutput").ap()

    with tile.TileContext(nc) as tc, ExitStack() as ctx:
        singles = ctx.enter_context(tc.tile_pool(name="singles", bufs=1))

        # ---- SBUF resident tensors ----
        w1Ts = singles.tile([128, 5, 4 * H], BF16)      # 2.6 MB
        w2Ts = singles.tile([128, 5, 4 * KS], BF16)     # 0.65 MB
        woTs = singles.tile([128, 2, VOCAB], BF16)      # 2.1 MB
        keyTs = singles.tile([128, BL * NTC, 128], BF16)  # 1 MB [k, (b,tc), t]
        vTs = singles.tile([128, BL * NTC, VS], BF16)   # 1 MB [t, (b,tc), v]
        histH = singles.tile([128, L * BL], BF16)       # 0.5 MB
        histC = singles.tile([128, L * BL], BF16)       # 0.5 MB
        b2S = singles.tile([128, 4 * BL], F32)
        onesb = singles.tile([128, 128], BF16)

        # states
        h1 = singles.tile([128, 4 * BL], BF16)   # 2*h1: [p, 8m+b], h=128m+p
        c1 = singles.tile([128, 4 * BL], F32)
        h2 = singles.tile([128, BL], BF16)       # 2*h2
        c2 = singles.tile([128, BL], F32)
        ctxT = singles.tile([128, BL], BF16)     # context.T (true scale)

        gembA = singles.tile([128, 128], F32)
        gembB = singles.tile([128, 128], F32)

        # ---- prologue: loads ----
        nc.sync.dma_start(w1Ts[:], d_w1T[:])
        nc.sync.dma_start(w2Ts[:], d_w2T[:])
        nc.sync.dma_start(woTs[:], d_woT[:])
        nc.sync.dma_start(keyTs[:], d_key[:])
        nc.sync.dma_start(vTs[:], d_val[:])
        nc.sync.dma_start(ctxT[:], d_v0[:])
        nc.sync.dma_start(b2S[:], d_b2[:])

        nc.vector.memset(onesb[:], 1.0)
        nc.vector.memset(h1[:], 0.0)
        nc.vector.memset(c1[:], 0.0)
        nc.vector.memset(h2[:], 0.0)
        nc.vector.memset(c2[:], 0.0)

        # ---- PSUM + temp pools ----
        loop_ctx = ctx.enter_context(ExitStack())
        ppool = loop_ctx.enter_context(tc.tile_pool(name="ppool", bufs=1, space="PSUM"))
        temps = loop_ctx.enter_context(tc.tile_pool(name="temps", bufs=2))

        pg1 = ppool.tile([128, 128], F32, tag="pg1")
        pg2 = ppool.tile([128, 4 * BL], F32, tag="pg2")
        pE = ppool.tile([128, BL * NTC], F32, tag="pE")
        pCtx = ppool.tile([128, BL], F32, tag="pCtx")
        pZ = ppool.tile([128, BL, NTC], F32, tag="pZ")

        def lstm1_mms(gemb_t):
            # g1.T in PSUM (128,128); col 8m+b = gate-row 128m+p
            # contraction chunks: kc0 = ctx (w1ctx), kc1..4 = h1 (w_hh1/2)
            for m in range(16):
                o = pg1[:, m * BL:(m + 1) * BL]
                for kc in range(5):
                    rhs = ctxT[:] if kc == 0 else h1[:, (kc - 1) * BL:kc * BL]
                    nc.tensor.matmul(o, w1Ts[:, kc, m * 128:(m + 1) * 128], rhs,
                                     start=(kc == 0), stop=(kc == 4))
            # add host-precomputed embedding-gate + bias tile (fp32, in psum)
            nc.vector.tensor_add(pg1[:], pg1[:], gemb_t)

        def gates1():
            tifo = temps.tile([128, 96], F32, tag="tifo")
            tg = temps.tile([128, 32], F32, tag="tg")
            nc.scalar.activation(tifo[:], pg1[:, 0:96], AF.Tanh, scale=0.5)
            nc.scalar.activation(tg[:], pg1[:, 96:128], AF.Tanh)
            sif = temps.tile([128, 64], F32, tag="sif")
            nc.vector.tensor_scalar(sif[:], tifo[:, 0:64], 0.5, 0.5,
                                    OP.mult, OP.add)
            a1 = temps.tile([128, 32], F32, tag="a1")
            nc.vector.tensor_mul(a1[:], sif[:, 0:32], tg[:])
            nc.vector.tensor_mul(c1[:], sif[:, 32:64], c1[:])
            nc.vector.tensor_add(c1[:], c1[:], a1[:])
            tc1 = temps.tile([128, 32], F32, tag="tc1")
            nc.scalar.activation(tc1[:], c1[:], AF.Tanh)
            # h1 state = 2*h1 = (tanh(o/2) + 1) * tanh(c)
            nc.vector.scalar_tensor_tensor(h1[:], tifo[:, 64:96], 1.0, tc1[:],
                                           OP.add, OP.mult)

        def lstm2_mms():
            # g2.T in PSUM (128,32); col 8m+b = gate-row 128m+p
            # chunks: kc0..3 = h1 (w_ih2/2), kc4 = h2 (w_hh2/2)
            for m in range(4):
                o = pg2[:, m * BL:(m + 1) * BL]
                for kc in range(5):
                    rhs = h1[:, kc * BL:(kc + 1) * BL] if kc < 4 else h2[:]
                    nc.tensor.matmul(o, w2Ts[:, kc, m * 128:(m + 1) * 128], rhs,
                                     start=(kc == 0), stop=(kc == 4))
            nc.vector.tensor_add(pg2[:], pg2[:], b2S[:])

        def gates2(t):
            tifo2 = temps.tile([128, 24], F32, tag="tifo2")
            tg2 = temps.tile([128, 8], F32, tag="tg2")
            nc.scalar.activation(tifo2[:], pg2[:, 0:24], AF.Tanh, scale=0.5)
            nc.scalar.activation(tg2[:], pg2[:, 24:32], AF.Tanh)
            sif2 = temps.tile([128, 16], F32, tag="sif2")
            nc.vector.tensor_scalar(sif2[:], tifo2[:, 0:16], 0.5, 0.5,
                                    OP.mult, OP.add)
            a2 = temps.tile([128, 8], F32, tag="a2")
            nc.vector.tensor_mul(a2[:], sif2[:, 0:8], tg2[:])
            nc.vector.tensor_mul(c2[:], sif2[:, 8:16], c2[:])
            nc.vector.tensor_add(c2[:], c2[:], a2[:])
            tc2 = temps.tile([128, 8], F32, tag="tc2")
            nc.scalar.activation(tc2[:], c2[:], AF.Tanh)
            nc.vector.scalar_tensor_tensor(h2[:], tifo2[:, 16:24], 1.0, tc2[:],
                                           OP.add, OP.mult)
            nc.gpsimd.tensor_copy(histH[:, ds(t * BL, BL)], h2[:])

        def attention(t):
            # energy columns: pE[:, b*4+tc] = keys[b,tc].T @ h2[:, b]
            for b in range(BL):
                for tcn in range(NTC):
                    c = b * NTC + tcn
                    nc.tensor.matmul(pE[:, c:c + 1],
                                     keyTs[:, c, :],
                                     h2[:, b:b + 1], start=True, stop=True)
            expS = temps.tile([128, BL * NTC], BF16, tag="expS")
            nc.scalar.activation(expS[:], pE[:], AF.Exp)
            # Z (replicated over partitions) via all-ones matmul
            nc.tensor.matmul(pZ[:, :, :], onesb[:], expS[:], start=True, stop=True)
            # context: pCtx[:, b] = sum_tc V[b,tc].T @ expS[:, b*4+tc]
            for b in range(BL):
                for tcn in range(NTC):
                    c = b * NTC + tcn
                    nc.tensor.matmul(pCtx[:, b:b + 1],
                                     vTs[:, c, :],
                                     expS[:, c:c + 1],
                                     start=(tcn == 0), stop=(tcn == NTC - 1))
            # Z[b] = sum_tc colsum(b,tc) - pad correction; rz = 1/Z
            zr = temps.tile([128, BL], F32, tag="zr")
            nc.vector.tensor_reduce(zr[:], pZ[:, :, :], mybir.AxisListType.X,
                                    OP.add)
            nc.vector.tensor_scalar_add(zr[:], zr[:], -GERR)
            rz = temps.tile([128, BL], F32, tag="rz")
            nc.vector.reciprocal(rz[:], zr[:])
            nc.vector.tensor_mul(ctxT[:], pCtx[:], rz[:])
            nc.gpsimd.tensor_copy(histC[:, ds(t * BL, BL)], ctxT[:])

        def step(t, gemb_t):
            lstm1_mms(gemb_t)
            gates1()
            lstm2_mms()
            gates2(t)
            attention(t)

        with tc.For_i(0, L) as t:
            nc.sync.dma_start(gembA[:], d_gemb[:, ds(t * 128, 128)])
            step(t, gembA[:])
        loop_ctx.close()

        # ===== deferred vocab projection (bias added on host) =====
        NB = 4
        nblk = (L * BL) // NB
        with tc.tile_pool(name="projp", bufs=4, space="PSUM") as projp, \
             tc.tile_pool(name="projs", bufs=4) as projs:
            for vc in range(VOCAB // 128):
                for nb in range(NB):
                    pp = projp.tile([128, nblk], F32, tag="pp")
                    sl = ds(nb * nblk, nblk)
                    nc.tensor.matmul(pp[:], woTs[:, 0, vc * 128:(vc + 1) * 128],
                                     histH[:, sl], start=True, stop=False)
                    nc.tensor.matmul(pp[:], woTs[:, 1, vc * 128:(vc + 1) * 128],
                                     histC[:, sl], start=False, stop=True)
                    ob = projs.tile([128, nblk], F32, tag="ob")
                    if nb % 2 == 0:
                        nc.vector.tensor_copy(ob[:], pp[:])
                    else:
                        nc.scalar.copy(ob[:], pp[:])
                    nc.sync.dma_start(d_out[vc][:, sl], ob[:])

    nc.compile()
    return nc


_CACHE = {}


def _get_nc(L):
    if L not in _CACHE:
        _CACHE[L] = build(L)
    return _CACHE[L]


def _bf16(x):
    import ml_dtypes
    return np.ascontiguousarray(x.astype(ml_dtypes.bfloat16))


def _prep_inputs(key, values, speech_len, text, embedding,
                 w_ih1, b_ih1, w_hh1, b_hh1,
                 w_ih2, b_ih2, w_hh2, b_hh2,
                 w_out, b_out, L):
    f = np.float32
    key = np.asarray(key, f)
    values = np.asarray(values, f)
    speech_len = np.asarray(speech_len)
    text = np.asarray(text)
    embedding = np.asarray(embedding, f)
    w_ih1 = np.asarray(w_ih1, f)
    w_hh1 = np.asarray(w_hh1, f)
    w_ih2 = np.asarray(w_ih2, f)
    w_hh2 = np.asarray(w_hh2, f)
    w_out = np.asarray(w_out, f)

    # gate reorder [i, f, g, o] -> [i, f, o, g]
    def reorder(w, hdim):
        blocks = w.reshape(4, hdim, -1) if w.ndim == 2 else w.reshape(4, hdim)
        return np.concatenate([blocks[0], blocks[1], blocks[3], blocks[2]],
                              axis=0)

    w_ih1r = reorder(w_ih1, H)
    w_hh1r = reorder(w_hh1, H)
    b1r = reorder(np.asarray(b_ih1, f) + np.asarray(b_hh1, f), H)
    w_ih2r = reorder(w_ih2, KS)
    w_hh2r = reorder(w_hh2, KS)
    b2r = reorder(np.asarray(b_ih2, f) + np.asarray(b_hh2, f), KS)

    # w1: chunks [ctx (x1), h1 (x1/2, since h1 state = 2*h1)] -> (5,128,4H)
    w1cat = np.concatenate([w_ih1r[:, E:E + VS], w_hh1r * 0.5], axis=1)  # (2048, 640)
    w1T = _bf16(w1cat.T.reshape(5, 128, 4 * H).transpose(1, 0, 2)
                .reshape(128, 5 * 4 * H))
    # w2: chunks [h1 (x1/2), h2 (x1/2)]
    w2cat = np.concatenate([w_ih2r * 0.5, w_hh2r * 0.5], axis=1)  # (512, 640)
    w2T = _bf16(w2cat.T.reshape(5, 128, 4 * KS).transpose(1, 0, 2)
                .reshape(128, 5 * 4 * KS))
    # w_out: [h2-part (x1/2), ctx-part (x1)]
    woS = np.concatenate([w_out[:, :KS] * 0.5, w_out[:, KS:]], axis=1)
    woT = _bf16(woS.T.reshape(2, 128, VOCAB).transpose(1, 0, 2)
                .reshape(128, 2 * VOCAB))
    # b2 tile [128, 4m x 8b]: gate 128m+p
    b2S = np.ascontiguousarray(
        np.broadcast_to(b2r.reshape(4, 128).T[:, :, None], (128, 4, BL))
        .reshape(128, 4 * BL).astype(f))

    # teacher-forcing tokens -> embeddings -> host-precomputed gate tiles
    tokens = np.concatenate(
        [np.zeros((B, 1), text.dtype), text[:, :L - 1]], axis=1)  # (B, L)
    embeds = embedding[tokens]  # (B, L, E)
    # gemb[b, t, :] = w_ih1r[:, :E] @ embeds[b, t] + b1r   (fp32 on host)
    gemb = embeds.reshape(B * L, E) @ w_ih1r[:, :E].T.astype(f)
    gemb += b1r[None, :]
    gemb = gemb.reshape(B, L, 4 * H)

    mask = (np.arange(T)[:, None] < np.asarray(speech_len)[None, :])  # (T, B)

    in_maps = []
    for c in range(NCORES):
        bs = slice(c * BL, (c + 1) * BL)
        # gembD [128 p, L t, 16 m, 8 b]
        gD = np.ascontiguousarray(
            gemb[bs].transpose(2, 1, 0)              # (2048, L, 8)
            .reshape(16, 128, L, BL).transpose(1, 2, 0, 3)  # (128, L, 16, 8)
            .reshape(128, L * 128).astype(f))
        # keys: masked, padded, halved (h2 state = 2*h2): [128 k, 8 b, 4 tc, 128 t]
        km = key[:, bs, :] * mask[:, bs, None].astype(f)  # (T, BL, KS)
        kp = np.zeros((TP, BL, KS), f)
        kp[:T] = km * 0.5
        kT = _bf16(kp.transpose(2, 1, 0).reshape(128, BL, NTC, 128)
                   .reshape(128, BL * NTC * 128))
        # values: padded: [128 t, 8 b, 4 tc, 128 v]
        vp = np.zeros((TP, BL, VS), f)
        vp[:T] = values[:, bs, :]
        vT = _bf16(vp.reshape(NTC, 128, BL, VS).transpose(1, 2, 0, 3)
                   .reshape(128, BL * NTC * VS))
        in_maps.append(dict(
            w1T=w1T, w2T=w2T, woT=woT,
            keyT=kT, valT=vT,
            val0T=_bf16(values[0, bs, :].T),
            b2S=b2S, gembD=gD))
    return in_maps


def kernel(key, values, speech_len, text, embedding,
           w_ih1, b_ih1, w_hh1, b_hh1,
           w_ih2, b_ih2, w_hh2, b_hh2,
           w_out, b_out, _L=250, _trace=False, _tmpdir=None):
    L = _L
    nc = _get_nc(L)
    in_maps = _prep_inputs(key, values, speech_len, text, embedding,
                           w_ih1, b_ih1, w_hh1, b_hh1,
                           w_ih2, b_ih2, w_hh2, b_hh2, w_out, b_out, L)
    kw = {}
    if _trace:
        kw = dict(trace=True, tmpdir=_tmpdir)
    res = run_bass_kernel_spmd(nc, in_maps, core_ids=list(range(NCORES)), **kw)
    kernel._last = res
    b_out = np.asarray(b_out, np.float32)
    out = np.empty((B, L, VOCAB), np.float32)
    for c in range(NCORES):
        p = res.results[c]["predT"]  # (32, 128, L*BL)
        out[c * BL:(c + 1) * BL] = (
            p.reshape(VOCAB // 128, 128, L, BL).transpose(3, 2, 0, 1)
            .reshape(BL, L, VOCAB))
    out += b_out[None, None, :]
    return out
